# revision 18
# baseline (speedup 1.0000x reference)
"""CAlphaUpdate kernel for 8 Trainium2 NeuronCores.

Math (reference):
  upd = concat([node @ Ws.T + bs, (rot @ ((node @ Wv.T + bv).reshape(-1,8,3)) + trans)])
  atom_features[CA rows] += upd (gated by atom_mask[:,1] & ca_select)
  out = irrep_batchnorm(atom_features)   # global stats over ALL atoms

Sharding: data-parallel over residues (8192 residues = 114688 atoms per core).
Batchnorm channel statistics (sum & sum-of-squares per channel) are computed
per-core with PE ones-matmuls and combined with a cross-core AllReduce.

Per-core layout tricks:
  * atoms viewed as [R, 14*56] -> one residue per SBUF partition row; the CA
    atom is a fixed free-dim slice (cols 56:112).
  * node shard pre-transposed on host to [257, R] (row 256 = per-residue gate,
    which also folds the bias via an augmented-K matmul).
  * W_big [257,104]: cols 0:32 scalar head; cols 32+24j+q (q=3v+i) replicate
    the vector head over i so the per-residue rotation becomes elementwise:
    psum rows 32+24j+q hold local[j,v]; rot_rep rows 24j+q hold rot[i,j];
    one [72,N] multiply + two [24,N] partition-offset adds finish rot apply.
  * Stats: sum(x) and sum(x^2) per channel via ones[128,1] stationary matmuls
    accumulated in PSUM over all tiles; CA-row correction uses the updated CA
    block (computed once, kept in SBUF).
  * Pass 2 applies y = A*x + D with A/D broadcast to [128,784] via K=1 matmul.
"""

import os
import sys

import numpy as np

sys.path.insert(0, "/opt/trn_rl_repo")

import concourse.bacc as bacc
import concourse.bass as bass
import concourse.tile as tile
from concourse import mybir
from concourse.masks import make_identity
from concourse import bass_utils

F32 = mybir.dt.float32
AF = mybir.ActivationFunctionType
ALU = mybir.AluOpType

N_CORES = 8
N_RES = 65536
APR = 14
NCH = 56  # 32 scalars + 8 vectors * 3
FREE = APR * NCH  # 784
C_NODE = 256
EPS = 1e-5
CHUNK = 512

_PROGRAM_CACHE = {}
LAST_RESULT = None


def build_program(R, n_cores):
    """Emit the per-core Bass program for R residues per core."""
    assert R % CHUNK == 0 and R % 128 == 0
    n_chunks = R // CHUNK
    n_tiles = R // 128
    inv_n = 1.0 / float(R * n_cores * APR)

    nc = bacc.Bacc("TRN2", target_bir_lowering=False, debug=False,
                   num_devices=n_cores)

    atoms = nc.dram_tensor("atoms", [R, FREE], F32, kind="ExternalInput")
    nodet = nc.dram_tensor("nodet", [C_NODE + 1, R], F32, kind="ExternalInput")
    rotrep = nc.dram_tensor("rotrep", [96, R], F32, kind="ExternalInput")
    transrep = nc.dram_tensor("transrep", [24, R], F32, kind="ExternalInput")
    wbig = nc.dram_tensor("wbig", [C_NODE + 1, 128], F32, kind="ExternalInput")
    sel = nc.dram_tensor("sel", [96, 24], F32, kind="ExternalInput")
    affw = nc.dram_tensor("affw", [1, NCH], F32, kind="ExternalInput")
    affb = nc.dram_tensor("affb", [1, NCH], F32, kind="ExternalInput")
    outa = nc.dram_tensor("outa", [R, FREE], F32, kind="ExternalOutput")

    with tile.TileContext(nc) as tc:
        with tc.tile_pool(name="singles", bufs=1) as singles:
            w0 = singles.tile([128, 128], F32)
            w1 = singles.tile([128, 128], F32)
            w2 = singles.tile([1, 128], F32)
            nc.sync.dma_start(out=w0, in_=wbig[0:128, :])
            nc.sync.dma_start(out=w1, in_=wbig[128:256, :])
            nc.sync.dma_start(out=w2, in_=wbig[256:257, :])
            ones_col = singles.tile([128, 1], F32)
            nc.vector.memset(ones_col, 1.0)
            ones_row = singles.tile([1, 128], F32)
            nc.vector.memset(ones_row, 1.0)
            ident32 = singles.tile([32, 32], F32)
            make_identity(nc, ident32)
            ident24 = singles.tile([24, 24], F32)
            make_identity(nc, ident24)
            sel_sb = singles.tile([96, 24], F32)
            nc.sync.dma_start(out=sel_sb, in_=sel[:, :])
            affw_sb = singles.tile([1, NCH], F32)
            affb_sb = singles.tile([1, NCH], F32)
            nc.sync.dma_start(out=affw_sb, in_=affw[:, :])
            nc.sync.dma_start(out=affb_sb, in_=affb[:, :])
            # updated-CA rows, transposed to [residue-on-partition, 56]
            updT = singles.tile([128, n_tiles * NCH], F32)

            # ---- Phase A: upd = W_big @ node_aug, rotation applied ----
            with tc.tile_pool(name="nodep", bufs=3) as nodep, \
                 tc.tile_pool(name="awork", bufs=3) as awork, \
                 tc.tile_pool(name="apsum", bufs=2, space="PSUM") as apsum, \
                 tc.tile_pool(name="trpsum", bufs=1, space="PSUM") as trpsum:
                for c in range(n_chunks):
                    s = c * CHUNK
                    na = nodep.tile([128, CHUNK], F32, tag="na")
                    nb = nodep.tile([128, CHUNK], F32, tag="nb")
                    ng = nodep.tile([1, CHUNK], F32, tag="ng")
                    rr = nodep.tile([96, CHUNK], F32, tag="rr")
                    tr = nodep.tile([24, CHUNK], F32, tag="tr")
                    nc.sync.dma_start(out=na, in_=nodet[0:128, s:s + CHUNK])
                    nc.sync.dma_start(out=nb, in_=nodet[128:256, s:s + CHUNK])
                    nc.sync.dma_start(out=ng, in_=nodet[256:257, s:s + CHUNK])
                    nc.sync.dma_start(out=rr, in_=rotrep[:, s:s + CHUNK])
                    nc.sync.dma_start(out=tr, in_=transrep[:, s:s + CHUNK])

                    pu_s = apsum.tile([32, CHUNK], F32, tag="pu_s")
                    pu_v = apsum.tile([96, CHUNK], F32, tag="pu_v")
                    nc.tensor.matmul(pu_s, w0[:, 0:32], na, start=True,
                                     stop=False)
                    nc.tensor.matmul(pu_s, w1[:, 0:32], nb, start=False,
                                     stop=False)
                    nc.tensor.matmul(pu_s, w2[:, 0:32], ng, start=False,
                                     stop=True)
                    nc.tensor.matmul(pu_v, w0[:, 32:128], na, start=True,
                                     stop=False)
                    nc.tensor.matmul(pu_v, w1[:, 32:128], nb, start=False,
                                     stop=False)
                    nc.tensor.matmul(pu_v, w2[:, 32:128], ng, start=False,
                                     stop=True)

                    upre_s = awork.tile([32, CHUNK], F32, tag="upre_s")
                    nc.scalar.activation(out=upre_s, in_=pu_s, func=AF.Copy)
                    m96 = awork.tile([96, CHUNK], F32, tag="m96")
                    nc.vector.tensor_mul(m96, pu_v, rr)
                    # reduce the three j-slabs on PE: pv[q] = sum_j m96[32j+q]
                    pv = apsum.tile([24, CHUNK], F32, tag="pv")
                    nc.tensor.matmul(pv, sel_sb, m96, start=True, stop=True)
                    upre_v = awork.tile([24, CHUNK], F32, tag="upre_v")
                    nc.vector.tensor_add(upre_v, pv, tr)

                    for k in range(CHUNK // 128):
                        pt = trpsum.tile([128, NCH], F32, tag="pt")
                        ksl = slice(k * 128, (k + 1) * 128)
                        nc.tensor.transpose(pt[:, 0:32], upre_s[:, ksl],
                                            ident32)
                        nc.tensor.transpose(pt[:, 32:56], upre_v[:, ksl],
                                            ident24)
                        col = (c * (CHUNK // 128) + k) * NCH
                        nc.scalar.activation(out=updT[:, col:col + NCH],
                                             in_=pt, func=AF.Copy)

            # ---- Phase B: channel statistics over this core's shard ----
            with tc.tile_pool(name="bload", bufs=4) as bload, \
                 tc.tile_pool(name="bwork", bufs=3) as bwork, \
                 tc.tile_pool(name="statps", bufs=1, space="PSUM") as statps:
                psA = statps.tile([1, FREE], F32)   # sum(x) per col
                psB = statps.tile([1, FREE], F32)   # sum(x^2) per col
                psC1 = statps.tile([1, 56], F32)    # updated-CA sum
                psC2 = statps.tile([1, 56], F32)    # updated-CA sumsq
                for t in range(n_tiles):
                    r0 = t * 128
                    first = (t == 0)
                    last = (t == n_tiles - 1)
                    at = bload.tile([128, FREE], F32, tag="at")
                    nc.sync.dma_start(out=at, in_=atoms[r0:r0 + 128, :])
                    sq = bwork.tile([128, FREE], F32, tag="sq")
                    nc.scalar.square(sq, at)
                    nc.tensor.matmul(psA[:, 0:512], ones_col, at[:, 0:512],
                                     start=first, stop=last)
                    nc.tensor.matmul(psA[:, 512:FREE], ones_col,
                                     at[:, 512:FREE], start=first, stop=last)
                    nc.tensor.matmul(psB[:, 0:512], ones_col, sq[:, 0:512],
                                     start=first, stop=last)
                    nc.tensor.matmul(psB[:, 512:FREE], ones_col,
                                     sq[:, 512:FREE], start=first, stop=last)
                    tca = bwork.tile([128, NCH], F32, tag="tca")
                    nc.vector.tensor_add(tca, at[:, 56:112],
                                         updT[:, t * NCH:(t + 1) * NCH])
                    sqca = bwork.tile([128, NCH], F32, tag="sqca")
                    nc.scalar.square(sqca, tca)
                    nc.tensor.matmul(psC1, ones_col, tca,
                                     start=first, stop=last)
                    nc.tensor.matmul(psC2, ones_col, sqca,
                                     start=first, stop=last)

                # ---- Phase C: reduce 14 atom groups, allreduce, coeffs ----
                sA = singles.tile([1, FREE], F32)
                sB = singles.tile([1, FREE], F32)
                sC = singles.tile([1, 112], F32)
                nc.scalar.activation(out=sA, in_=psA, func=AF.Copy)
                nc.scalar.activation(out=sB, in_=psB, func=AF.Copy)
                nc.scalar.activation(out=sC[:, 0:56], in_=psC1, func=AF.Copy)
                nc.scalar.activation(out=sC[:, 56:112], in_=psC2, func=AF.Copy)

            S = singles.tile([1, 112], F32)
            # sum over atom groups, excluding original CA (group 1), plus
            # the updated-CA contribution.
            nc.vector.tensor_add(S[:, 0:56], sA[:, 0:56], sA[:, 112:168])
            nc.vector.tensor_add(S[:, 56:112], sB[:, 0:56], sB[:, 112:168])
            for a in range(3, APR):
                nc.vector.tensor_add(S[:, 0:56], S[:, 0:56],
                                     sA[:, a * 56:(a + 1) * 56])
                nc.vector.tensor_add(S[:, 56:112], S[:, 56:112],
                                     sB[:, a * 56:(a + 1) * 56])
            nc.vector.tensor_add(S[:, 0:56], S[:, 0:56], sC[:, 0:56])
            nc.vector.tensor_add(S[:, 56:112], S[:, 56:112], sC[:, 56:112])

            g = singles.tile([1, 112], F32)
            if n_cores > 1:
                with tc.tile_pool(name="ccdram", bufs=1, space="DRAM") as ccd:
                    cci = ccd.tile([1, 112], F32)
                    cco = ccd.tile([1, 112], F32)
                    nc.sync.dma_start(out=cci, in_=S)
                    nc.gpsimd.collective_compute(
                        "AllReduce", ALU.add,
                        replica_groups=[list(range(n_cores))],
                        ins=[cci.opt()], outs=[cco.opt()])
                    nc.sync.dma_start(out=g, in_=cco)
            else:
                nc.vector.tensor_copy(g, S)

            mean = singles.tile([1, NCH], F32)
            nc.vector.tensor_scalar_mul(mean, g[:, 0:56], inv_n)
            ex2 = singles.tile([1, NCH], F32)
            nc.vector.tensor_scalar_mul(ex2, g[:, 56:112], inv_n)
            # vector channels (cols 32:56): no mean subtraction
            nc.vector.memset(mean[:, 32:56], 0.0)
            msq = singles.tile([1, NCH], F32)
            nc.vector.tensor_mul(msq, mean, mean)
            den = singles.tile([1, NCH], F32)
            nc.vector.tensor_sub(den, ex2, msq)
            # vector norm: mean over the 3 components, replicated back
            dv = den[:, 32:56].rearrange("p (v t) -> p v t", t=3)
            vsum = singles.tile([1, 8], F32)
            nc.vector.tensor_add(vsum, dv[:, :, 0], dv[:, :, 1])
            nc.vector.tensor_add(vsum, vsum, dv[:, :, 2])
            nc.vector.tensor_scalar_mul(vsum, vsum, 1.0 / 3.0)
            for i in range(3):
                nc.vector.tensor_copy(dv[:, :, i], vsum)
            eps_sb = singles.tile([1, 1], F32)
            nc.vector.memset(eps_sb, EPS)
            sd = singles.tile([1, NCH], F32)
            nc.scalar.activation(out=sd, in_=den, func=AF.Sqrt, bias=eps_sb)
            rinv = singles.tile([1, NCH], F32)
            nc.vector.reciprocal(rinv, sd)
            A56 = singles.tile([1, NCH], F32)
            nc.vector.tensor_mul(A56, rinv, affw_sb)
            D56 = singles.tile([1, NCH], F32)
            nc.vector.tensor_mul(D56, mean, A56)
            nc.vector.tensor_sub(D56, affb_sb, D56)
            A784 = singles.tile([1, FREE], F32)
            D784 = singles.tile([1, FREE], F32)
            for a in range(APR):
                nc.vector.tensor_copy(A784[:, a * 56:(a + 1) * 56], A56)
                nc.vector.tensor_copy(D784[:, a * 56:(a + 1) * 56], D56)

            Abc = singles.tile([128, FREE], F32)
            Dbc = singles.tile([128, FREE], F32)
            with tc.tile_pool(name="bcps", bufs=1, space="PSUM") as bcps:
                pbc = bcps.tile([128, FREE], F32, tag="pbc")
                nc.tensor.matmul(pbc[:, 0:512], ones_row, A784[:, 0:512],
                                 start=True, stop=True)
                nc.tensor.matmul(pbc[:, 512:FREE], ones_row, A784[:, 512:FREE],
                                 start=True, stop=True)
                nc.scalar.activation(out=Abc, in_=pbc, func=AF.Copy)
                pbc2 = bcps.tile([128, FREE], F32, tag="pbc2")
                nc.tensor.matmul(pbc2[:, 0:512], ones_row, D784[:, 0:512],
                                 start=True, stop=True)
                nc.tensor.matmul(pbc2[:, 512:FREE], ones_row,
                                 D784[:, 512:FREE], start=True, stop=True)
                nc.scalar.activation(out=Dbc, in_=pbc2, func=AF.Copy)

            # ---- Phase D: y = A*(x + CA update) + D ----
            with tc.tile_pool(name="dload", bufs=4) as dload:
                for t in range(n_tiles):
                    r0 = t * 128
                    at2 = dload.tile([128, FREE], F32, tag="at2")
                    nc.sync.dma_start(out=at2, in_=atoms[r0:r0 + 128, :])
                    nc.vector.tensor_add(at2[:, 56:112], at2[:, 56:112],
                                         updT[:, t * NCH:(t + 1) * NCH])
                    nc.vector.tensor_mul(at2, at2, Abc)
                    nc.vector.tensor_add(at2, at2, Dbc)
                    nc.sync.dma_start(out=outa[r0:r0 + 128, :], in_=at2)

    nc.compile()
    return nc


def prepare_full(inputs, n_res):
    """Host-side global prep: fold gate/bias, build replicated layouts."""
    af = np.ascontiguousarray(
        np.asarray(inputs["atom_features"], dtype=np.float32)
    ).reshape(n_res, FREE)
    node = np.asarray(inputs["node_features"], dtype=np.float32)
    rot = np.asarray(inputs["rot"], dtype=np.float32)
    trans = np.asarray(inputs["trans"], dtype=np.float32)
    ca = np.asarray(inputs["ca_select"]).reshape(n_res, APR)
    am = np.asarray(inputs["atom_mask"], dtype=np.float32)
    Ws = np.asarray(inputs["Ws"], dtype=np.float32)
    bs = np.asarray(inputs["bs"], dtype=np.float32)
    Wv = np.asarray(inputs["Wv"], dtype=np.float32)
    bv = np.asarray(inputs["bv"], dtype=np.float32)
    bnw = np.asarray(inputs["bn_weight"], dtype=np.float32)
    bnb = np.asarray(inputs["bn_bias"], dtype=np.float32)

    gate = ((am[:, 1] > 0.5) & (ca[:, 1] != 0)).astype(np.float32)

    wbig = np.zeros((C_NODE + 1, 128), np.float32)
    wbig[:C_NODE, 0:32] = Ws.T
    wbig[C_NODE, 0:32] = bs
    for j in range(3):
        idx = np.repeat(3 * np.arange(8) + j, 3)  # col q=3v+i -> Wv row 3v+j
        wbig[:C_NODE, 32 + 32 * j:56 + 32 * j] = Wv[idx].T
        wbig[C_NODE, 32 + 32 * j:56 + 32 * j] = bv[idx]

    nodet = np.empty((C_NODE + 1, n_res), np.float32)
    np.multiply(node.T, gate[None, :], out=nodet[:C_NODE])
    nodet[C_NODE] = gate

    rotrep = np.zeros((96, n_res), np.float32)
    for j in range(3):
        rotrep[32 * j:32 * j + 24] = np.tile(rot[:, :, j].T, (8, 1))
    transrep = np.tile(trans.T * gate[None, :], (8, 1)).astype(np.float32)

    affw = np.concatenate([bnw[:32], np.repeat(bnw[32:40], 3)])
    affw = np.ascontiguousarray(affw.reshape(1, NCH), dtype=np.float32)
    affb = np.concatenate([bnb, np.zeros(24, np.float32)])
    affb = np.ascontiguousarray(affb.reshape(1, NCH), dtype=np.float32)
    sel = np.zeros((96, 24), np.float32)
    for j in range(3):
        sel[32 * j + np.arange(24), np.arange(24)] = 1.0
    return af, nodet, rotrep, transrep, wbig, affw, affb, sel


def make_in_maps(inputs, n_res, n_cores):
    (af, nodet, rotrep, transrep, wbig, affw, affb, sel) = prepare_full(
        inputs, n_res)
    R = n_res // n_cores
    in_maps = []
    for c in range(n_cores):
        sl = slice(c * R, (c + 1) * R)
        in_maps.append({
            "atoms": np.ascontiguousarray(af[sl]),
            "nodet": np.ascontiguousarray(nodet[:, sl]),
            "rotrep": np.ascontiguousarray(rotrep[:, sl]),
            "transrep": np.ascontiguousarray(transrep[:, sl]),
            "wbig": wbig,
            "affw": affw,
            "affb": affb,
            "sel": sel,
        })
    return in_maps


def kernel(**inputs):
    global LAST_RESULT
    R = N_RES // N_CORES
    key = (R, N_CORES)
    if key not in _PROGRAM_CACHE:
        _PROGRAM_CACHE[key] = build_program(R, N_CORES)
    nc = _PROGRAM_CACHE[key]
    in_maps = make_in_maps(inputs, N_RES, N_CORES)
    trace = bool(int(os.environ.get("KERNEL_TRACE", "0")))
    if trace:
        try:  # NTFF profiling needs the axon hook; absent in some containers
            from antenv.axon_hooks import get_axon_ntff_profile_hook  # noqa
        except ImportError:
            trace = False
    res = bass_utils.run_bass_kernel_spmd(nc, in_maps, list(range(N_CORES)),
                                          trace=trace)
    LAST_RESULT = res
    out = np.concatenate([res.results[c]["outa"] for c in range(N_CORES)],
                         axis=0)
    return np.ascontiguousarray(out.reshape(N_RES * APR, NCH))


# revision 20
# speedup vs baseline: 1.0052x; 1.0052x over previous
"""CAlphaUpdate kernel for 8 Trainium2 NeuronCores.

Math (reference):
  upd = concat([node @ Ws.T + bs, rot @ ((node @ Wv.T + bv).reshape(-1,8,3)) + trans])
  atom_features[CA rows] += upd (gated by atom_mask[:,1] & ca_select)
  out = irrep_batchnorm(atom_features)   # global stats over ALL atoms

Sharding: data-parallel over residues (8192 residues = 114688 atoms per core).
Batchnorm channel statistics (sum & sum-of-squares per channel) are computed
per-core and combined with a cross-core AllReduce.

Per-core structure (R=8192 residues, atoms viewed [R, 14*56]):
  A) update compute: bf16 matmul W_big.T @ node_aug -> PSUM fp32;
     per-residue rotation as one elementwise [96,512] multiply against a
     replicated-rot layout + a PE selection-matmul that also folds trans;
     PE-transposed into updT [residue, 56], SBUF-resident.
  B) stats: per-channel sum/sumsq via ones-stationary bf16 matmuls
     accumulated in PSUM over all tiles; CA rows corrected with updated
     values. First KRES atom tiles stay SBUF-resident for phase D.
  C) 14-atom-group reduce, 8-core AllReduce, batchnorm affine coeffs,
     broadcast A/D rows to [128, 784] via K=1 matmul.
  D) y = A*(x + CA update) + D streamed back out (resident tiles skip the
     second HBM read).
"""

import os
import sys

import numpy as np

sys.path.insert(0, "/opt/trn_rl_repo")

import ml_dtypes

import concourse.bacc as bacc
import concourse.bass as bass
import concourse.tile as tile
from concourse import mybir
from concourse.masks import make_identity
from concourse import bass_utils

F32 = mybir.dt.float32
BF16 = mybir.dt.bfloat16
AF = mybir.ActivationFunctionType
ALU = mybir.AluOpType
NP_BF16 = ml_dtypes.bfloat16

N_CORES = 8
N_RES = 65536
APR = 14
NCH = 56  # 32 scalars + 8 vectors * 3
FREE = APR * NCH  # 784
C_NODE = 256
EPS = 1e-5
CHUNK = 512
KRES_MAX = 30  # atom tiles kept SBUF-resident between phases B and D

_PROGRAM_CACHE = {}
LAST_RESULT = None


def build_program(R, n_cores):
    """Emit the per-core Bass program for R residues per core."""
    assert R % CHUNK == 0 and R % 128 == 0
    n_chunks = R // CHUNK
    n_tiles = R // 128
    kres = min(KRES_MAX, n_tiles)
    inv_n = 1.0 / float(R * n_cores * APR)

    nc = bacc.Bacc("TRN2", target_bir_lowering=False, debug=False,
                   num_devices=n_cores)

    atoms = nc.dram_tensor("atoms", [R, FREE], F32, kind="ExternalInput")
    nodet = nc.dram_tensor("nodet", [C_NODE + 1, R], F32,
                           kind="ExternalInput")
    rotrep = nc.dram_tensor("rotrep", [96, R], F32, kind="ExternalInput")
    trans3 = nc.dram_tensor("trans3", [3, R], F32, kind="ExternalInput")
    wbig = nc.dram_tensor("wbig", [C_NODE + 1, 128], F32,
                          kind="ExternalInput")
    sel = nc.dram_tensor("sel", [99, 24], F32, kind="ExternalInput")
    affw = nc.dram_tensor("affw", [1, NCH], F32, kind="ExternalInput")
    affb = nc.dram_tensor("affb", [1, NCH], F32, kind="ExternalInput")
    outa = nc.dram_tensor("outa", [R, FREE], F32, kind="ExternalOutput")

    with tile.TileContext(nc) as tc:
        with tc.tile_pool(name="singles", bufs=1) as singles:
            w0 = singles.tile([128, 128], F32)
            w1 = singles.tile([128, 128], F32)
            w2 = singles.tile([1, 128], F32)
            nc.sync.dma_start(out=w0, in_=wbig[0:128, :])
            nc.sync.dma_start(out=w1, in_=wbig[128:256, :])
            nc.sync.dma_start(out=w2, in_=wbig[256:257, :])
            ones_b = singles.tile([128, 1], BF16)
            nc.vector.memset(ones_b, 1.0)
            ones_f = singles.tile([128, 1], F32)
            nc.vector.memset(ones_f, 1.0)
            ones_row = singles.tile([1, 128], F32)
            nc.vector.memset(ones_row, 1.0)
            ident32 = singles.tile([32, 32], F32)
            make_identity(nc, ident32)
            ident24 = singles.tile([24, 24], F32)
            make_identity(nc, ident24)
            sel_sb = singles.tile([99, 24], F32)
            nc.sync.dma_start(out=sel_sb, in_=sel[:, :])
            affw_sb = singles.tile([1, NCH], F32)
            affb_sb = singles.tile([1, NCH], F32)
            nc.sync.dma_start(out=affw_sb, in_=affw[:, :])
            nc.sync.dma_start(out=affb_sb, in_=affb[:, :])
            # updated-CA rows, transposed to [residue-on-partition, 56]
            updT = singles.tile([128, n_tiles * NCH], F32)
            # SBUF-resident atom tiles reused between phases B and D
            res_tiles = [
                singles.tile([128, FREE], F32, name=f"resid{t}",
                             tag=f"resid{t}")
                for t in range(kres)
            ]

            # ---- Phase A: upd = W_big @ node_aug, rotation applied ----
            with tc.tile_pool(name="nodep", bufs=3) as nodep, \
                 tc.tile_pool(name="awork", bufs=3) as awork, \
                 tc.tile_pool(name="apsum", bufs=2, space="PSUM") as apsum, \
                 tc.tile_pool(name="trpsum", bufs=2, space="PSUM") as trpsum:
                for c in range(n_chunks):
                    s = c * CHUNK
                    na = nodep.tile([128, CHUNK], F32, tag="na")
                    nb = nodep.tile([128, CHUNK], F32, tag="nb")
                    ng = nodep.tile([1, CHUNK], F32, tag="ng")
                    rr = nodep.tile([96, CHUNK], F32, tag="rr")
                    nc.sync.dma_start(out=na, in_=nodet[0:128, s:s + CHUNK])
                    nc.sync.dma_start(out=nb, in_=nodet[128:256, s:s + CHUNK])
                    nc.sync.dma_start(out=ng, in_=nodet[256:257, s:s + CHUNK])
                    nc.sync.dma_start(out=rr, in_=rotrep[:, s:s + CHUNK])

                    pu_s = apsum.tile([32, CHUNK], F32, tag="pu_s")
                    pu_v = apsum.tile([96, CHUNK], F32, tag="pu_v")
                    nc.tensor.matmul(pu_s, w0[:, 0:32], na, start=True,
                                     stop=False)
                    nc.tensor.matmul(pu_s, w1[:, 0:32], nb, start=False,
                                     stop=False)
                    nc.tensor.matmul(pu_s, w2[:, 0:32], ng, start=False,
                                     stop=True)
                    nc.tensor.matmul(pu_v, w0[:, 32:128], na, start=True,
                                     stop=False)
                    nc.tensor.matmul(pu_v, w1[:, 32:128], nb, start=False,
                                     stop=False)
                    nc.tensor.matmul(pu_v, w2[:, 32:128], ng, start=False,
                                     stop=True)

                    upre_s = awork.tile([32, CHUNK], F32, tag="upre_s")
                    nc.scalar.activation(out=upre_s, in_=pu_s, func=AF.Copy)
                    # m99 rows 0:96: rot[i,j] * local[j,v]; rows 96:99: trans
                    m99 = awork.tile([99, CHUNK], F32, tag="m99")
                    nc.vector.tensor_mul(m99[0:96, :], pu_v, rr)
                    nc.sync.dma_start(out=m99[96:99, :],
                                      in_=trans3[:, s:s + CHUNK])
                    # PE reduce: pv[q] = sum_j m99[32j+q] + trans[i(q)]
                    pv = apsum.tile([24, CHUNK], F32, tag="pv")
                    nc.tensor.matmul(pv, sel_sb, m99, start=True, stop=True)
                    upre_v = awork.tile([24, CHUNK], F32, tag="upre_v")
                    nc.scalar.activation(out=upre_v, in_=pv, func=AF.Copy)

                    for k in range(CHUNK // 128):
                        pt = trpsum.tile([128, NCH], F32, tag="pt")
                        ksl = slice(k * 128, (k + 1) * 128)
                        nc.tensor.transpose(pt[:, 0:32], upre_s[:, ksl],
                                            ident32)
                        nc.tensor.transpose(pt[:, 32:56], upre_v[:, ksl],
                                            ident24)
                        col = (c * (CHUNK // 128) + k) * NCH
                        nc.scalar.activation(out=updT[:, col:col + NCH],
                                             in_=pt, func=AF.Copy)

            # ---- Phase B: channel statistics over this core's shard ----
            with tc.tile_pool(name="bload", bufs=4) as bload, \
                 tc.tile_pool(name="bwork", bufs=3) as bwork, \
                 tc.tile_pool(name="statps", bufs=1, space="PSUM") as statps:
                psA = statps.tile([1, FREE], F32)   # sum(x) per col
                psB = statps.tile([1, FREE], F32)   # sum(x^2) per col
                psC1 = statps.tile([1, 56], F32)    # updated-CA sum
                psC2 = statps.tile([1, 56], F32)    # updated-CA sumsq
                for t in range(n_tiles):
                    r0 = t * 128
                    first = (t == 0)
                    last = (t == n_tiles - 1)
                    if t < kres:
                        at = res_tiles[t]
                    else:
                        at = bload.tile([128, FREE], F32, tag="at")
                    nc.sync.dma_start(out=at, in_=atoms[r0:r0 + 128, :])
                    xb = bwork.tile([128, FREE], BF16, tag="xb")
                    nc.gpsimd.tensor_copy(xb, at)
                    sqb = bwork.tile([128, FREE], BF16, tag="sqb")
                    nc.scalar.square(sqb, at)
                    nc.tensor.matmul(psA[:, 0:512], ones_b, xb[:, 0:512],
                                     start=first, stop=last)
                    nc.tensor.matmul(psA[:, 512:FREE], ones_b,
                                     xb[:, 512:FREE], start=first, stop=last)
                    nc.tensor.matmul(psB[:, 0:512], ones_b, sqb[:, 0:512],
                                     start=first, stop=last)
                    nc.tensor.matmul(psB[:, 512:FREE], ones_b,
                                     sqb[:, 512:FREE], start=first, stop=last)
                    tca = bwork.tile([128, NCH], BF16, tag="tca")
                    nc.vector.tensor_add(tca, at[:, 56:112],
                                         updT[:, t * NCH:(t + 1) * NCH])
                    sqca = bwork.tile([128, NCH], BF16, tag="sqca")
                    nc.scalar.square(sqca, tca)
                    nc.tensor.matmul(psC1, ones_b, tca,
                                     start=first, stop=last)
                    nc.tensor.matmul(psC2, ones_b, sqca,
                                     start=first, stop=last)

                # ---- Phase C: reduce 14 atom groups, allreduce, coeffs ----
                sA = singles.tile([1, FREE], F32)
                sB = singles.tile([1, FREE], F32)
                sC = singles.tile([1, 112], F32)
                nc.scalar.activation(out=sA, in_=psA, func=AF.Copy)
                nc.scalar.activation(out=sB, in_=psB, func=AF.Copy)
                nc.scalar.activation(out=sC[:, 0:56], in_=psC1, func=AF.Copy)
                nc.scalar.activation(out=sC[:, 56:112], in_=psC2,
                                     func=AF.Copy)

            S = singles.tile([1, 112], F32)
            # sum over atom groups, excluding original CA (group 1), plus
            # the updated-CA contribution.
            nc.vector.tensor_add(S[:, 0:56], sA[:, 0:56], sA[:, 112:168])
            nc.vector.tensor_add(S[:, 56:112], sB[:, 0:56], sB[:, 112:168])
            for a in range(3, APR):
                nc.vector.tensor_add(S[:, 0:56], S[:, 0:56],
                                     sA[:, a * 56:(a + 1) * 56])
                nc.vector.tensor_add(S[:, 56:112], S[:, 56:112],
                                     sB[:, a * 56:(a + 1) * 56])
            nc.vector.tensor_add(S[:, 0:56], S[:, 0:56], sC[:, 0:56])
            nc.vector.tensor_add(S[:, 56:112], S[:, 56:112], sC[:, 56:112])

            g = singles.tile([1, 112], F32)
            if n_cores > 1:
                with tc.tile_pool(name="ccdram", bufs=1, space="DRAM") as ccd:
                    cci = ccd.tile([1, 112], F32)
                    cco = ccd.tile([1, 112], F32)
                    nc.sync.dma_start(out=cci, in_=S)
                    nc.gpsimd.collective_compute(
                        "AllReduce", ALU.add,
                        replica_groups=[list(range(n_cores))],
                        ins=[cci.opt()], outs=[cco.opt()])
                    nc.sync.dma_start(out=g, in_=cco)
            else:
                nc.vector.tensor_copy(g, S)

            mean = singles.tile([1, NCH], F32)
            nc.vector.tensor_scalar_mul(mean, g[:, 0:56], inv_n)
            ex2 = singles.tile([1, NCH], F32)
            nc.vector.tensor_scalar_mul(ex2, g[:, 56:112], inv_n)
            # vector channels (cols 32:56): no mean subtraction
            nc.vector.memset(mean[:, 32:56], 0.0)
            msq = singles.tile([1, NCH], F32)
            nc.vector.tensor_mul(msq, mean, mean)
            den = singles.tile([1, NCH], F32)
            nc.vector.tensor_sub(den, ex2, msq)
            # vector norm: mean over the 3 components, replicated back
            dv = den[:, 32:56].rearrange("p (v t) -> p v t", t=3)
            vsum = singles.tile([1, 8], F32)
            nc.vector.tensor_add(vsum, dv[:, :, 0], dv[:, :, 1])
            nc.vector.tensor_add(vsum, vsum, dv[:, :, 2])
            nc.vector.tensor_scalar_mul(vsum, vsum, 1.0 / 3.0)
            for i in range(3):
                nc.vector.tensor_copy(dv[:, :, i], vsum)
            eps_sb = singles.tile([1, 1], F32)
            nc.vector.memset(eps_sb, EPS)
            sd = singles.tile([1, NCH], F32)
            nc.scalar.activation(out=sd, in_=den, func=AF.Sqrt, bias=eps_sb)
            rinv = singles.tile([1, NCH], F32)
            nc.vector.reciprocal(rinv, sd)
            A56 = singles.tile([1, NCH], F32)
            nc.vector.tensor_mul(A56, rinv, affw_sb)
            D56 = singles.tile([1, NCH], F32)
            nc.vector.tensor_mul(D56, mean, A56)
            nc.vector.tensor_sub(D56, affb_sb, D56)
            A784 = singles.tile([1, FREE], F32)
            D784 = singles.tile([1, FREE], F32)
            for a in range(APR):
                nc.vector.tensor_copy(A784[:, a * 56:(a + 1) * 56], A56)
                nc.vector.tensor_copy(D784[:, a * 56:(a + 1) * 56], D56)

            Abc = singles.tile([128, FREE], F32)
            Dbc = singles.tile([128, FREE], F32)
            with tc.tile_pool(name="bcps", bufs=1, space="PSUM") as bcps:
                pbc = bcps.tile([128, FREE], F32, tag="pbc")
                nc.tensor.matmul(pbc[:, 0:512], ones_row, A784[:, 0:512],
                                 start=True, stop=True)
                nc.tensor.matmul(pbc[:, 512:FREE], ones_row, A784[:, 512:FREE],
                                 start=True, stop=True)
                nc.scalar.activation(out=Abc, in_=pbc, func=AF.Copy)
                pbc2 = bcps.tile([128, FREE], F32, tag="pbc2")
                nc.tensor.matmul(pbc2[:, 0:512], ones_row, D784[:, 0:512],
                                 start=True, stop=True)
                nc.tensor.matmul(pbc2[:, 512:FREE], ones_row,
                                 D784[:, 512:FREE], start=True, stop=True)
                nc.scalar.activation(out=Dbc, in_=pbc2, func=AF.Copy)

            # ---- Phase D: y = A*(x + CA update) + D ----
            with tc.tile_pool(name="dload", bufs=4) as dload:
                for t in range(n_tiles):
                    r0 = t * 128
                    if t < kres:
                        at2 = res_tiles[t]
                    else:
                        at2 = dload.tile([128, FREE], F32, tag="at2")
                        nc.sync.dma_start(out=at2, in_=atoms[r0:r0 + 128, :])
                    nc.vector.tensor_add(at2[:, 56:112], at2[:, 56:112],
                                         updT[:, t * NCH:(t + 1) * NCH])
                    nc.vector.tensor_mul(at2, at2, Abc)
                    nc.vector.tensor_add(at2, at2, Dbc)
                    nc.sync.dma_start(out=outa[r0:r0 + 128, :], in_=at2)

    nc.compile()
    return nc


def prepare_full(inputs, n_res):
    """Host-side global prep: fold gate/bias, build replicated layouts."""
    af = np.ascontiguousarray(
        np.asarray(inputs["atom_features"], dtype=np.float32)
    ).reshape(n_res, FREE)
    node = np.asarray(inputs["node_features"], dtype=np.float32)
    rot = np.asarray(inputs["rot"], dtype=np.float32)
    trans = np.asarray(inputs["trans"], dtype=np.float32)
    ca = np.asarray(inputs["ca_select"]).reshape(n_res, APR)
    am = np.asarray(inputs["atom_mask"], dtype=np.float32)
    Ws = np.asarray(inputs["Ws"], dtype=np.float32)
    bs = np.asarray(inputs["bs"], dtype=np.float32)
    Wv = np.asarray(inputs["Wv"], dtype=np.float32)
    bv = np.asarray(inputs["bv"], dtype=np.float32)
    bnw = np.asarray(inputs["bn_weight"], dtype=np.float32)
    bnb = np.asarray(inputs["bn_bias"], dtype=np.float32)

    gate = ((am[:, 1] > 0.5) & (ca[:, 1] != 0)).astype(np.float32)

    wbig = np.zeros((C_NODE + 1, 128), np.float32)
    wbig[:C_NODE, 0:32] = Ws.T
    wbig[C_NODE, 0:32] = bs
    for j in range(3):
        idx = np.repeat(3 * np.arange(8) + j, 3)  # col q=3v+i -> Wv row 3v+j
        wbig[:C_NODE, 32 + 32 * j:56 + 32 * j] = Wv[idx].T
        wbig[C_NODE, 32 + 32 * j:56 + 32 * j] = bv[idx]

    nodet = np.empty((C_NODE + 1, n_res), np.float32)
    np.multiply(node.T, gate[None, :], out=nodet[:C_NODE])
    nodet[C_NODE] = gate

    rotrep = np.zeros((96, n_res), np.float32)
    for j in range(3):
        rotrep[32 * j:32 * j + 24] = np.tile(rot[:, :, j].T, (8, 1))
    trans3 = np.ascontiguousarray(trans.T * gate[None, :], dtype=np.float32)

    affw = np.concatenate([bnw[:32], np.repeat(bnw[32:40], 3)])
    affw = np.ascontiguousarray(affw.reshape(1, NCH), dtype=np.float32)
    affb = np.concatenate([bnb, np.zeros(24, np.float32)])
    affb = np.ascontiguousarray(affb.reshape(1, NCH), dtype=np.float32)
    # selection matrix: rows 32j+q pick slab j (q=3v+i); rows 96+i add trans
    sel = np.zeros((99, 24), np.float32)
    for j in range(3):
        sel[32 * j + np.arange(24), np.arange(24)] = 1.0
    for i in range(3):
        sel[96 + i, 3 * np.arange(8) + i] = 1.0
    return af, nodet, rotrep, trans3, wbig, affw, affb, sel


def make_in_maps(inputs, n_res, n_cores):
    (af, nodet, rotrep, trans3, wbig, affw, affb, sel) = prepare_full(
        inputs, n_res)
    R = n_res // n_cores
    in_maps = []
    for c in range(n_cores):
        sl = slice(c * R, (c + 1) * R)
        in_maps.append({
            "atoms": np.ascontiguousarray(af[sl]),
            "nodet": np.ascontiguousarray(nodet[:, sl]),
            "rotrep": np.ascontiguousarray(rotrep[:, sl]),
            "trans3": np.ascontiguousarray(trans3[:, sl]),
            "wbig": wbig,
            "affw": affw,
            "affb": affb,
            "sel": sel,
        })
    return in_maps


def kernel(**inputs):
    global LAST_RESULT
    R = N_RES // N_CORES
    key = (R, N_CORES)
    if key not in _PROGRAM_CACHE:
        _PROGRAM_CACHE[key] = build_program(R, N_CORES)
    nc = _PROGRAM_CACHE[key]
    in_maps = make_in_maps(inputs, N_RES, N_CORES)
    trace = bool(int(os.environ.get("KERNEL_TRACE", "0")))
    if trace:
        try:  # NTFF profiling needs the axon hook; absent in some containers
            from antenv.axon_hooks import get_axon_ntff_profile_hook  # noqa
        except ImportError:
            trace = False
    res = bass_utils.run_bass_kernel_spmd(nc, in_maps, list(range(N_CORES)),
                                          trace=trace)
    LAST_RESULT = res
    out = np.concatenate([res.results[c]["outa"] for c in range(N_CORES)],
                         axis=0)
    return np.ascontiguousarray(out.reshape(N_RES * APR, NCH))


# revision 31
# speedup vs baseline: 1.6520x; 1.6435x over previous
"""CAlphaUpdate kernel for 8 Trainium2 NeuronCores.

Math (reference):
  upd = concat([node @ Ws.T + bs, rot @ ((node @ Wv.T + bv).reshape(-1,8,3)) + trans])
  atom_features[CA rows] += upd (gated by atom_mask[:,1] & ca_select)
  out = irrep_batchnorm(atom_features)   # global stats over ALL atoms

Sharding: data-parallel over residues (8192 residues = 114688 atoms per core).
Batchnorm channel statistics (sum & sum-of-squares per channel) are computed
per-core and combined with a cross-core AllReduce.

Per-core structure (R=8192 residues, atoms viewed [R, 14*56]):
  A) update compute: bf16 matmul W_big.T @ node_aug -> PSUM fp32;
     per-residue rotation as one elementwise [96,512] multiply against a
     replicated-rot layout + a PE selection-matmul that also folds trans;
     PE-transposed into updT [residue, 56], SBUF-resident.
  B) stats: per-channel sum/sumsq via ones-stationary bf16 matmuls
     accumulated in PSUM over all tiles; CA rows corrected with updated
     values. First KRES atom tiles stay SBUF-resident for phase D.
  C) 14-atom-group reduce, 8-core AllReduce, batchnorm affine coeffs,
     broadcast A/D rows to [128, 784] via K=1 matmul.
  D) y = A*(x + CA update) + D streamed back out (resident tiles skip the
     second HBM read).
"""

import os
import sys

import numpy as np

sys.path.insert(0, "/opt/trn_rl_repo")

import ml_dtypes

import concourse.bacc as bacc
import concourse.bass as bass
import concourse.tile as tile
from concourse import mybir
from concourse.masks import make_identity
from concourse import bass_utils

F32 = mybir.dt.float32
BF16 = mybir.dt.bfloat16
FP16 = mybir.dt.float16
AF = mybir.ActivationFunctionType
ALU = mybir.AluOpType
NP_BF16 = ml_dtypes.bfloat16

N_CORES = 8
N_RES = 65536
APR = 14
NCH = 56  # 32 scalars + 8 vectors * 3
FREE = APR * NCH  # 784
C_NODE = 256
EPS = 1e-5
CHUNK = 512
KRES_MAX = 30  # atom tiles kept SBUF-resident between phases B and D

_PROGRAM_CACHE = {}
LAST_RESULT = None


def build_program(R, n_cores):
    """Emit the per-core Bass program for R residues per core."""
    assert R % CHUNK == 0 and R % 128 == 0
    n_chunks = R // CHUNK
    n_tiles = R // 128
    kres = min(KRES_MAX, n_tiles)
    if (n_tiles - kres) % 2:
        kres -= 1
    inv_n = 1.0 / float(R * n_cores * APR)

    nc = bacc.Bacc("TRN2", target_bir_lowering=False, debug=False,
                   num_devices=n_cores)

    atoms = nc.dram_tensor("atoms", [R, FREE], F32, kind="ExternalInput")
    nodet = nc.dram_tensor("nodet", [C_NODE + 1, R], F32,
                           kind="ExternalInput")
    rotrep = nc.dram_tensor("rotrep", [96, R], F32, kind="ExternalInput")
    trans3 = nc.dram_tensor("trans3", [3, R], F32, kind="ExternalInput")
    wbig = nc.dram_tensor("wbig", [C_NODE + 1, 128], F32,
                          kind="ExternalInput")
    sel = nc.dram_tensor("sel", [99, 24], F32, kind="ExternalInput")
    affw = nc.dram_tensor("affw", [1, NCH], F32, kind="ExternalInput")
    affb = nc.dram_tensor("affb", [1, NCH], F32, kind="ExternalInput")
    outa = nc.dram_tensor("outa", [R, FREE], F32, kind="ExternalOutput")

    with tile.TileContext(nc) as tc:
        with tc.tile_pool(name="singles", bufs=1) as singles:
            w0 = singles.tile([128, 128], F32)
            w1 = singles.tile([128, 128], F32)
            w2 = singles.tile([1, 128], F32)
            nc.sync.dma_start(out=w0, in_=wbig[0:128, :])
            nc.sync.dma_start(out=w1, in_=wbig[128:256, :])
            nc.sync.dma_start(out=w2, in_=wbig[256:257, :])
            ones_b = singles.tile([128, 1], BF16)
            nc.vector.memset(ones_b, 1.0)
            ones_f = singles.tile([128, 1], F32)
            nc.vector.memset(ones_f, 1.0)
            ones_row = singles.tile([1, 128], F32)
            nc.vector.memset(ones_row, 1.0)
            ident32 = singles.tile([32, 32], F32)
            make_identity(nc, ident32)
            ident24 = singles.tile([24, 24], F32)
            make_identity(nc, ident24)
            sel_sb = singles.tile([99, 24], F32)
            nc.sync.dma_start(out=sel_sb, in_=sel[:, :])
            affw_sb = singles.tile([1, NCH], F32)
            affb_sb = singles.tile([1, NCH], F32)
            nc.sync.dma_start(out=affw_sb, in_=affw[:, :])
            nc.sync.dma_start(out=affb_sb, in_=affb[:, :])
            # updated-CA rows, transposed to [residue-on-partition, 56]
            updT = singles.tile([128, n_tiles * NCH], F32)
            # SBUF-resident atom tiles reused between phases B and D
            res_tiles = [
                singles.tile([128, FREE], F32, name=f"resid{t}",
                             tag=f"resid{t}")
                for t in range(kres)
            ]

            # ---- Phase A: upd = W_big @ node_aug, rotation applied ----
            with tc.tile_pool(name="nodep", bufs=3) as nodep, \
                 tc.tile_pool(name="awork", bufs=3) as awork, \
                 tc.tile_pool(name="apsum", bufs=2, space="PSUM") as apsum, \
                 tc.tile_pool(name="trpsum", bufs=2, space="PSUM") as trpsum:
                for c in range(n_chunks):
                    s = c * CHUNK
                    na = nodep.tile([128, CHUNK], F32, tag="na")
                    nb = nodep.tile([128, CHUNK], F32, tag="nb")
                    ng = nodep.tile([1, CHUNK], F32, tag="ng")
                    rr = nodep.tile([96, CHUNK], F32, tag="rr")
                    nc.sync.dma_start(out=na, in_=nodet[0:128, s:s + CHUNK])
                    nc.sync.dma_start(out=nb, in_=nodet[128:256, s:s + CHUNK])
                    nc.sync.dma_start(out=ng,
                                        in_=nodet[256:257, s:s + CHUNK])
                    nc.sync.dma_start(out=rr, in_=rotrep[:, s:s + CHUNK])

                    pu_s = apsum.tile([32, CHUNK], F32, tag="pu_s")
                    pu_v = apsum.tile([96, CHUNK], F32, tag="pu_v")
                    nc.tensor.matmul(pu_s, w0[:, 0:32], na, start=True,
                                     stop=False)
                    nc.tensor.matmul(pu_s, w1[:, 0:32], nb, start=False,
                                     stop=False)
                    nc.tensor.matmul(pu_s, w2[:, 0:32], ng, start=False,
                                     stop=True)
                    nc.tensor.matmul(pu_v, w0[:, 32:128], na, start=True,
                                     stop=False)
                    nc.tensor.matmul(pu_v, w1[:, 32:128], nb, start=False,
                                     stop=False)
                    nc.tensor.matmul(pu_v, w2[:, 32:128], ng, start=False,
                                     stop=True)

                    upre_s = awork.tile([32, CHUNK], F32, tag="upre_s")
                    nc.scalar.activation(out=upre_s, in_=pu_s, func=AF.Copy)
                    # m99 rows 0:96: rot[i,j] * local[j,v]; rows 96:99: trans
                    m99 = awork.tile([99, CHUNK], F32, tag="m99")
                    nc.vector.tensor_mul(m99[0:96, :], pu_v, rr)
                    nc.sync.dma_start(out=m99[96:99, :],
                                        in_=trans3[:, s:s + CHUNK])
                    # PE reduce: pv[q] = sum_j m99[32j+q] + trans[i(q)]
                    pv = apsum.tile([24, CHUNK], F32, tag="pv")
                    nc.tensor.matmul(pv, sel_sb, m99, start=True, stop=True)
                    upre_v = awork.tile([24, CHUNK], F32, tag="upre_v")
                    nc.scalar.activation(out=upre_v, in_=pv, func=AF.Copy)

                    for k in range(CHUNK // 128):
                        pt = trpsum.tile([128, NCH], F32, tag="pt")
                        ksl = slice(k * 128, (k + 1) * 128)
                        nc.tensor.transpose(pt[:, 0:32], upre_s[:, ksl],
                                            ident32)
                        nc.tensor.transpose(pt[:, 32:56], upre_v[:, ksl],
                                            ident24)
                        col = (c * (CHUNK // 128) + k) * NCH
                        nc.scalar.activation(out=updT[:, col:col + NCH],
                                             in_=pt, func=AF.Copy)

            # ---- Phase B: channel statistics over this core's shard ----
            with tc.tile_pool(name="bload", bufs=4) as bload, \
                 tc.tile_pool(name="bwork", bufs=3) as bwork, \
                 tc.tile_pool(name="statps", bufs=1, space="PSUM") as statps:
                psA = statps.tile([1, FREE], F32)   # sum(x) per col
                psB = statps.tile([1, FREE], F32)   # sum(x^2) per col
                psC1 = statps.tile([1, 56], F32)    # updated-CA sum
                psC2 = statps.tile([1, 56], F32)    # updated-CA sumsq
                for t in range(n_tiles):
                    r0 = t * 128
                    first = (t == 0)
                    last = (t == n_tiles - 1)
                    if t < kres:
                        at = res_tiles[t]
                    else:
                        at = bload.tile([128, FREE], F32, tag="at")
                    nc.sync.dma_start(out=at, in_=atoms[r0:r0 + 128, :])
                    xb = bwork.tile([128, FREE], BF16, tag="xb")
                    nc.gpsimd.tensor_copy(xb, at)
                    sqb = bwork.tile([128, FREE], BF16, tag="sqb")
                    nc.scalar.square(sqb, at)
                    nc.tensor.matmul(psA[:, 0:512], ones_b, xb[:, 0:512],
                                     start=first, stop=last)
                    nc.tensor.matmul(psA[:, 512:FREE], ones_b,
                                     xb[:, 512:FREE], start=first, stop=last)
                    nc.tensor.matmul(psB[:, 0:512], ones_b, sqb[:, 0:512],
                                     start=first, stop=last)
                    nc.tensor.matmul(psB[:, 512:FREE], ones_b,
                                     sqb[:, 512:FREE], start=first, stop=last)
                    tca = bwork.tile([128, NCH], BF16, tag="tca")
                    nc.vector.tensor_add(tca, at[:, 56:112],
                                         updT[:, t * NCH:(t + 1) * NCH])
                    sqca = bwork.tile([128, NCH], BF16, tag="sqca")
                    nc.scalar.square(sqca, tca)
                    nc.tensor.matmul(psC1, ones_b, tca,
                                     start=first, stop=last)
                    nc.tensor.matmul(psC2, ones_b, sqca,
                                     start=first, stop=last)

                # ---- Phase C: reduce 14 atom groups, allreduce, coeffs ----
                sA = singles.tile([1, FREE], F32)
                sB = singles.tile([1, FREE], F32)
                sC = singles.tile([1, 112], F32)
                nc.scalar.activation(out=sA, in_=psA, func=AF.Copy)
                nc.scalar.activation(out=sB, in_=psB, func=AF.Copy)
                nc.scalar.activation(out=sC[:, 0:56], in_=psC1, func=AF.Copy)
                nc.scalar.activation(out=sC[:, 56:112], in_=psC2,
                                     func=AF.Copy)

            S = singles.tile([1, 112], F32)
            # sum over atom groups, excluding original CA (group 1), plus
            # the updated-CA contribution.
            nc.vector.tensor_add(S[:, 0:56], sA[:, 0:56], sA[:, 112:168])
            nc.vector.tensor_add(S[:, 56:112], sB[:, 0:56], sB[:, 112:168])
            for a in range(3, APR):
                nc.vector.tensor_add(S[:, 0:56], S[:, 0:56],
                                     sA[:, a * 56:(a + 1) * 56])
                nc.vector.tensor_add(S[:, 56:112], S[:, 56:112],
                                     sB[:, a * 56:(a + 1) * 56])
            nc.vector.tensor_add(S[:, 0:56], S[:, 0:56], sC[:, 0:56])
            nc.vector.tensor_add(S[:, 56:112], S[:, 56:112], sC[:, 56:112])

            g = singles.tile([1, 112], F32)
            if n_cores > 1:
                with tc.tile_pool(name="ccdram", bufs=1, space="DRAM") as ccd:
                    cci = ccd.tile([1, 112], F32)
                    cco = ccd.tile([1, 112], F32)
                    nc.sync.dma_start(out=cci, in_=S)
                    nc.gpsimd.collective_compute(
                        "AllReduce", ALU.add,
                        replica_groups=[list(range(n_cores))],
                        ins=[cci.opt()], outs=[cco.opt()])
                    nc.sync.dma_start(out=g, in_=cco)
            else:
                nc.vector.tensor_copy(g, S)

            mean = singles.tile([1, NCH], F32)
            nc.vector.tensor_scalar_mul(mean, g[:, 0:56], inv_n)
            ex2 = singles.tile([1, NCH], F32)
            nc.vector.tensor_scalar_mul(ex2, g[:, 56:112], inv_n)
            # vector channels (cols 32:56): no mean subtraction
            nc.vector.memset(mean[:, 32:56], 0.0)
            msq = singles.tile([1, NCH], F32)
            nc.vector.tensor_mul(msq, mean, mean)
            den = singles.tile([1, NCH], F32)
            nc.vector.tensor_sub(den, ex2, msq)
            # vector norm: mean over the 3 components, replicated back
            dv = den[:, 32:56].rearrange("p (v t) -> p v t", t=3)
            vsum = singles.tile([1, 8], F32)
            nc.vector.tensor_add(vsum, dv[:, :, 0], dv[:, :, 1])
            nc.vector.tensor_add(vsum, vsum, dv[:, :, 2])
            nc.vector.tensor_scalar_mul(vsum, vsum, 1.0 / 3.0)
            for i in range(3):
                nc.vector.tensor_copy(dv[:, :, i], vsum)
            eps_sb = singles.tile([1, 1], F32)
            nc.vector.memset(eps_sb, EPS)
            sd = singles.tile([1, NCH], F32)
            nc.scalar.activation(out=sd, in_=den, func=AF.Sqrt, bias=eps_sb)
            rinv = singles.tile([1, NCH], F32)
            nc.vector.reciprocal(rinv, sd)
            A56 = singles.tile([1, NCH], F32)
            nc.vector.tensor_mul(A56, rinv, affw_sb)
            D56 = singles.tile([1, NCH], F32)
            nc.vector.tensor_mul(D56, mean, A56)
            nc.vector.tensor_sub(D56, affb_sb, D56)
            A784 = singles.tile([1, FREE], F32)
            D784 = singles.tile([1, FREE], F32)
            for a in range(APR):
                nc.vector.tensor_copy(A784[:, a * 56:(a + 1) * 56], A56)
                nc.vector.tensor_copy(D784[:, a * 56:(a + 1) * 56], D56)

            Abc = singles.tile([128, FREE], F32)
            Dbc = singles.tile([128, FREE], F32)
            with tc.tile_pool(name="bcps", bufs=1, space="PSUM") as bcps:
                pbc = bcps.tile([128, FREE], F32, tag="pbc")
                nc.tensor.matmul(pbc[:, 0:512], ones_row, A784[:, 0:512],
                                 start=True, stop=True)
                nc.tensor.matmul(pbc[:, 512:FREE], ones_row, A784[:, 512:FREE],
                                 start=True, stop=True)
                nc.scalar.activation(out=Abc, in_=pbc, func=AF.Copy)
                pbc2 = bcps.tile([128, FREE], F32, tag="pbc2")
                nc.tensor.matmul(pbc2[:, 0:512], ones_row, D784[:, 0:512],
                                 start=True, stop=True)
                nc.tensor.matmul(pbc2[:, 512:FREE], ones_row,
                                 D784[:, 512:FREE], start=True, stop=True)
                nc.scalar.activation(out=Dbc, in_=pbc2, func=AF.Copy)

            # ---- Phase D: y = A*(x + CA update) + D ----
            with tc.tile_pool(name="dload", bufs=4) as dload:
                for t in range(n_tiles):
                    r0 = t * 128
                    if t < kres:
                        at2 = res_tiles[t]
                    else:
                        at2 = dload.tile([128, FREE], F32, tag="at2")
                        nc.sync.dma_start(out=at2, in_=atoms[r0:r0 + 128, :])
                    nc.vector.tensor_add(at2[:, 56:112], at2[:, 56:112],
                                         updT[:, t * NCH:(t + 1) * NCH])
                    nc.vector.tensor_mul(at2, at2, Abc)
                    nc.vector.tensor_add(at2, at2, Dbc)
                    nc.sync.dma_start(out=outa[r0:r0 + 128, :], in_=at2)

    nc.compile()
    return nc


def prepare_full(inputs, n_res):
    """Host-side global prep: fold gate/bias, build replicated layouts."""
    af = np.ascontiguousarray(
        np.asarray(inputs["atom_features"], dtype=np.float32)
    ).reshape(n_res, FREE)
    node = np.asarray(inputs["node_features"], dtype=np.float32)
    rot = np.asarray(inputs["rot"], dtype=np.float32)
    trans = np.asarray(inputs["trans"], dtype=np.float32)
    ca = np.asarray(inputs["ca_select"]).reshape(n_res, APR)
    am = np.asarray(inputs["atom_mask"], dtype=np.float32)
    Ws = np.asarray(inputs["Ws"], dtype=np.float32)
    bs = np.asarray(inputs["bs"], dtype=np.float32)
    Wv = np.asarray(inputs["Wv"], dtype=np.float32)
    bv = np.asarray(inputs["bv"], dtype=np.float32)
    bnw = np.asarray(inputs["bn_weight"], dtype=np.float32)
    bnb = np.asarray(inputs["bn_bias"], dtype=np.float32)

    gate = ((am[:, 1] > 0.5) & (ca[:, 1] != 0)).astype(np.float32)

    wbig = np.zeros((C_NODE + 1, 128), np.float32)
    wbig[:C_NODE, 0:32] = Ws.T
    wbig[C_NODE, 0:32] = bs
    for j in range(3):
        idx = np.repeat(3 * np.arange(8) + j, 3)  # col q=3v+i -> Wv row 3v+j
        wbig[:C_NODE, 32 + 32 * j:56 + 32 * j] = Wv[idx].T
        wbig[C_NODE, 32 + 32 * j:56 + 32 * j] = bv[idx]

    nodet = np.empty((C_NODE + 1, n_res), np.float32)
    np.multiply(node.T, gate[None, :], out=nodet[:C_NODE])
    nodet[C_NODE] = gate

    rotrep = np.zeros((96, n_res), np.float32)
    for j in range(3):
        rotrep[32 * j:32 * j + 24] = np.tile(rot[:, :, j].T, (8, 1))
    trans3 = np.ascontiguousarray(trans.T * gate[None, :], dtype=np.float32)

    affw = np.concatenate([bnw[:32], np.repeat(bnw[32:40], 3)])
    affw = np.ascontiguousarray(affw.reshape(1, NCH), dtype=np.float32)
    affb = np.concatenate([bnb, np.zeros(24, np.float32)])
    affb = np.ascontiguousarray(affb.reshape(1, NCH), dtype=np.float32)
    # selection matrix: rows 32j+q pick slab j (q=3v+i); rows 96+i add trans
    sel = np.zeros((99, 24), np.float32)
    for j in range(3):
        sel[32 * j + np.arange(24), np.arange(24)] = 1.0
    for i in range(3):
        sel[96 + i, 3 * np.arange(8) + i] = 1.0
    return af, nodet, rotrep, trans3, wbig, affw, affb, sel


def make_in_maps(inputs, n_res, n_cores):
    (af, nodet, rotrep, trans3, wbig, affw, affb, sel) = prepare_full(
        inputs, n_res)
    R = n_res // n_cores
    in_maps = []
    for c in range(n_cores):
        sl = slice(c * R, (c + 1) * R)
        in_maps.append({
            "atoms": np.ascontiguousarray(af[sl]),
            "nodet": np.ascontiguousarray(nodet[:, sl]),
            "rotrep": np.ascontiguousarray(rotrep[:, sl]),
            "trans3": np.ascontiguousarray(trans3[:, sl]),
            "wbig": wbig,
            "affw": affw,
            "affb": affb,
            "sel": sel,
        })
    return in_maps


def kernel(**inputs):
    global LAST_RESULT
    R = N_RES // N_CORES
    key = (R, N_CORES)
    if key not in _PROGRAM_CACHE:
        _PROGRAM_CACHE[key] = build_program(R, N_CORES)
    nc = _PROGRAM_CACHE[key]
    in_maps = make_in_maps(inputs, N_RES, N_CORES)
    trace = bool(int(os.environ.get("KERNEL_TRACE", "0")))
    if trace:
        try:  # NTFF profiling needs the axon hook; absent in some containers
            from antenv.axon_hooks import get_axon_ntff_profile_hook  # noqa
        except ImportError:
            trace = False
    res = bass_utils.run_bass_kernel_spmd(nc, in_maps, list(range(N_CORES)),
                                          trace=trace)
    LAST_RESULT = res
    out = np.concatenate([res.results[c]["outa"] for c in range(N_CORES)],
                         axis=0)
    return np.ascontiguousarray(out.reshape(N_RES * APR, NCH))
